# revision 42
# baseline (speedup 1.0000x reference)
"""Graph-Transformer message-passing kernel for 8 Trainium2 NeuronCores.

Strategy (1D dst-shard, fp8e3 qv table, pipelined SWDGE gathers):
  - Dst nodes are DEALT to the 8 cores by degree rank so all cores see
    near-identical degree profiles; each core owns all in-edges of its nodes,
    so segment softmax/aggregation are fully local.  Per-tile edge-slot
    widths are the 8-core max, kept tight by the dealing.
  - int16 gather indices force a lo/hi source-table split (<=32768 rows
    each); the lo source SET is chosen by greedy discrepancy minimization so
    every dst's in-degree splits ~50/50, collapsing the binomial width
    padding (~13% fewer gather slots).
  - Per layer, each core projects Q|K|V (PE), writes q|v scaled+clamped to
    fp8e3 (per-layer scales folded into host-side weights), AllGathers the
    table in 6 chunks overlapped with projection, then bulk-gathers source
    rows with dma_gather in prepare_only mode: Q7 desc-gen (the kernel's
    bottleneck, ~12ns/edge) retires without blocking on the SDMA drain, 4
    SWDGE queues round-robin, and explicit per-queue DMA-completion
    semaphores guard the consumers (Tile's own DMASW tracking is unsound for
    multi-queue/prep gathers).  lo-half gathers run a few groups ahead so
    desc-gen covers the hi chunks' AllGather tail.
  - Gathered fp8 rows are upconverted on ACT; masked edge softmax and the
    weighted aggregation run on DVE with tree reductions; output projection
    on PE.  3 layers fuse into one NEFF; the host inverts the permutation.
"""

import os
import numpy as np

import concourse.bass as bass
import concourse.bacc as bacc
import concourse.mybir as mybir
import concourse.tile as tile
from concourse.instruction_name_ordered_set import InstructionNameOrderedSet
from concourse.masks import make_identity
from concourse.bass_utils import run_bass_kernel_spmd

NCORES = 8
L = 3
H = 8
D = 128
HD = D // H
SCALE = 1.0 / float(np.sqrt(HD))
P = 128
NCH = 6              # AllGather chunks per layer
GROUP_COLS_CAP = 64  # max slot-columns (tiles*width) per gather group
GROUP_WASTE_CAP = 0.06
DEN_EPS = 6e-5
NQ = 4               # SWDGE queues for gather desc-gen / drain pipelining
ONE_AG = bool(int(os.environ.get("K_ONE_AG", "0")))
SKIP_EDGE = bool(int(os.environ.get("K_SKIP_EDGE", "0")))
GATHER_ONLY = bool(int(os.environ.get("K_GATHER_ONLY", "0")))

FP = mybir.dt.float32
F16 = mybir.dt.float16
F8 = mybir.dt.float8e3
I16 = mybir.dt.int16
AX = mybir.AxisListType
OP = mybir.AluOpType

# The q|v table is stored fp8e3 (1.9% RMS quantization) to halve the
# AllGather and gather-drain bytes.  Per-layer scales keep values in
# e3m4's [0.25, 15.5] sweet spot; they fold into the host-side weights
# (q: into wq/bq, undone via wk; v: into wv/bv, undone via wo).
F8_MAX = 15.5
SQ = [4.0, 16.0, 32.0]
SV = [4.0, 16.0, 32.0]


# ----------------------------------------------------------------------------
# Host-side layout
# ----------------------------------------------------------------------------

class Layout:
    pass


class Group:
    pass


def _wrap_idx16(flat):
    """int16 idx list -> [128, n/16] wrapped layout (16-partition, replicated)."""
    n = len(flat)
    assert n % 16 == 0
    arr = np.asarray(flat, dtype=np.int16).reshape(n // 16, 16).T  # [16, n/16]
    return np.tile(arr, (8, 1))  # [128, n/16]


def build_layout(src, dst, n_nodes):
    src = np.asarray(src).astype(np.int64)
    dst = np.asarray(dst).astype(np.int64)
    N = n_nodes
    chunk = (N + NCORES - 1) // NCORES
    T = (chunk + P - 1) // P
    R = T * P

    # AllGather tile chunks; lo/hi row split is at the boundary after chunk 1.
    # Within each half, the last chunk is kept small so the serial tail
    # (last out-proj -> proj -> AG -> first gather) is short.
    lo_tiles_n = (T + 1) // 2
    hi_tiles_n = T - lo_tiles_n
    def _split_lo(n):
        return [(n + 1) // 2, n - (n + 1) // 2]
    def _split_hi(n):
        # decreasing sizes: the last chunk gates the next layer's gathers
        a = max(1, (n * 4) // 10)
        b = max(1, (n * 3) // 10)
        c = max(1, (n * 2) // 10)
        return [a, b, c, n - a - b - c]
    tiles_per_ag = _split_lo(lo_tiles_n) + _split_hi(hi_tiles_n)
    tile_start = np.cumsum([0] + tiles_per_ag)  # len NCH+1
    CR = [tiles_per_ag[j] * P for j in range(NCH)]
    agbase = np.cumsum([0] + [NCORES * c for c in CR])  # global row base per chunk
    t_split = tile_start[2]          # tiles in the lo half (first two chunks)
    LO_POS = t_split * P             # per-core positions in lo half
    LO_ROWS = int(agbase[2])         # global rows in lo half
    TOT_ROWS = int(agbase[NCH])
    assert LO_ROWS <= 32768 and TOT_ROWS - LO_ROWS <= 32768

    def grow_of(core, pos):
        t = pos // P
        j = int(np.searchsorted(tile_start, t, side="right")) - 1
        return int(agbase[j]) + core * CR[j] + (pos - tile_start[j] * P)

    # --- choose the lo source set to BALANCE every dst's lo/hi in-degree ---
    # The int16 gather indices force a 2-table (lo/hi) source split; a random
    # split costs binomial variance in the per-tile widths.  Greedy
    # discrepancy minimization keeps dlo ~= dhi per dst, collapsing that
    # variance (and with it ~13% of the gather slots = Q7 desc-gen time).
    NLO = LO_POS * NCORES
    e_order = np.argsort(src, kind="stable")
    dst_by_src = dst[e_order]
    odeg = np.bincount(src, minlength=N)
    ostart = np.zeros(N + 1, dtype=np.int64)
    np.cumsum(odeg, out=ostart[1:])
    pot = np.zeros(N, dtype=np.int64)   # 2*dlo_assigned - deg_assigned
    is_lo = np.zeros(N, dtype=bool)
    quota, remaining = NLO, N
    for s in np.argsort(-odeg, kind="stable"):
        ds = dst_by_src[ostart[s]:ostart[s + 1]]
        p = pot[ds]
        go_lo = np.abs(p + 1).sum() < np.abs(p - 1).sum()
        if quota == 0:
            go_lo = False
        elif quota == remaining:
            go_lo = True
        if go_lo:
            is_lo[s] = True
            pot[ds] += 1
            quota -= 1
        else:
            pot[ds] -= 1
        remaining -= 1
    assert int(is_lo.sum()) == NLO
    edge_is_lo = is_lo[src]

    dlo = np.bincount(dst[edge_is_lo], minlength=N)
    dhi = np.bincount(dst[~edge_is_lo], minlength=N)

    # --- deal nodes to cores by degree rank (all cores see near-identical
    # degree profiles, so the 8-core per-tile width max stays tight);
    # auto-tune the sort key for minimal padding ---
    los = np.nonzero(is_lo)[0]
    his = np.nonzero(~is_lo)[0]
    keysets = [
        lambda g: np.lexsort((dhi[g], np.maximum(dlo[g], dhi[g]))),
        lambda g: np.lexsort((dlo[g], np.maximum(dlo[g], dhi[g]))),
        lambda g: np.lexsort((np.minimum(dlo[g], dhi[g]),
                              np.maximum(dlo[g], dhi[g]))),
        lambda g: np.lexsort((dhi[g] - dlo[g], dlo[g] + dhi[g])),
    ]
    best = None
    for key in keysets:
        perm_k = np.full((NCORES, R), -1, dtype=np.int64)
        lo_sorted = los[key(los)]
        hi_sorted = his[key(his)]
        for c in range(NCORES):
            sel = lo_sorted[c::NCORES]
            perm_k[c, : len(sel)] = sel
            # hi half descending: the layer's last group stays light, which
            # shortens the serial tail into the next layer's AllGather
            selh = hi_sorted[c::NCORES][::-1]
            perm_k[c, LO_POS: LO_POS + len(selh)] = selh
        Tn = R // P
        cost = 0
        for t in range(Tn):
            seg = perm_k[:, t * P:(t + 1) * P].reshape(-1)
            seg = seg[seg >= 0]
            if len(seg):
                cost += dlo[seg].max() + dhi[seg].max()
        if best is None or cost < best[0]:
            best = (cost, perm_k)
    perm = best[1]

    row_of = np.full(N, -1, dtype=np.int64)
    pos_of = np.full(N, -1, dtype=np.int64)
    for c in range(NCORES):
        real = perm[c] >= 0
        pp = np.nonzero(real)[0]
        nodes = perm[c][pp]
        pos_of[nodes] = pp
        row_of[nodes] = [grow_of(c, int(p)) for p in pp]
    row_of_arr = row_of

    # --- per-tile widths (max over cores) ---
    wlo_t = np.zeros(T, dtype=np.int64)
    whi_t = np.zeros(T, dtype=np.int64)
    for c in range(NCORES):
        for t in range(T):
            seg = perm[c, t * P:(t + 1) * P]
            seg = seg[seg >= 0]
            if len(seg):
                wlo_t[t] = max(wlo_t[t], dlo[seg].max())
                whi_t[t] = max(whi_t[t], dhi[seg].max())

    # --- group consecutive non-empty tiles ---
    groups = []
    w0_runs = []  # (nt0, ntiles) with W == 0
    t = 0
    gidx = 0
    while t < T:
        if wlo_t[t] + whi_t[t] == 0:
            t0 = t
            while t < T and wlo_t[t] + whi_t[t] == 0:
                t += 1
            w0_runs.append((t0, t - t0))
            continue
        g = Group()
        g.nt0 = t
        g.wlo = int(wlo_t[t])
        g.whi = int(whi_t[t])
        g.tiles = 1
        exact = g.wlo + g.whi
        t += 1
        while t < T and wlo_t[t] + whi_t[t] > 0:
            nw_lo = max(g.wlo, int(wlo_t[t]))
            nw_hi = max(g.whi, int(whi_t[t]))
            ncols = (g.tiles + 1) * (nw_lo + nw_hi)
            nexact = exact + int(wlo_t[t] + whi_t[t])
            if ncols > GROUP_COLS_CAP or (ncols - nexact) > GROUP_WASTE_CAP * nexact:
                break
            g.wlo, g.whi, g.tiles, exact = nw_lo, nw_hi, g.tiles + 1, nexact
            t += 1
        g.idx = gidx
        gidx += 1
        groups.append(g)

    # --- per-core idx / mask tables ---
    order = np.argsort(dst, kind="stable")
    src_sorted = src[order]
    deg = np.bincount(dst, minlength=N)
    starts = np.zeros(N + 1, dtype=np.int64)
    np.cumsum(deg, out=starts[1:])

    idx_cols = []
    mask_cols = 0
    for g in groups:
        g.ioff_lo = sum(idx_cols)
        idx_cols.append(8 * g.tiles * g.wlo)
        g.ioff_hi = sum(idx_cols)
        idx_cols.append(8 * g.tiles * g.whi)
        g.moff = mask_cols
        mask_cols += g.tiles * (g.wlo + g.whi)
    IC = max(sum(idx_cols), 16)
    MC = max(mask_cols, 1)

    idx_tabs = []
    mask_tabs = []
    for c in range(NCORES):
        itab = np.zeros((P, IC), dtype=np.int16)
        mtab = np.zeros((P, MC), dtype=np.float16)
        for g in groups:
            W = g.wlo + g.whi
            flat_lo = np.zeros(g.tiles * g.wlo * P, dtype=np.int16)
            flat_hi = np.zeros(g.tiles * g.whi * P, dtype=np.int16)
            for ti in range(g.tiles):
                tt = g.nt0 + ti
                for pp in range(P):
                    node = perm[c, tt * P + pp]
                    if node < 0:
                        continue
                    es = src_sorted[starts[node]: starts[node] + deg[node]]
                    es_rows = row_of_arr[es]
                    lo_rows = np.sort(es_rows[es_rows < LO_ROWS])
                    hi_rows = np.sort(es_rows[es_rows >= LO_ROWS]) - LO_ROWS
                    assert len(lo_rows) <= g.wlo and len(hi_rows) <= g.whi
                    for w, r in enumerate(lo_rows):
                        flat_lo[(ti * g.wlo + w) * P + pp] = r
                        mtab[pp, g.moff + ti * W + w] = 1.0
                    for w, r in enumerate(hi_rows):
                        flat_hi[(ti * g.whi + w) * P + pp] = r
                        mtab[pp, g.moff + ti * W + g.wlo + w] = 1.0
            if g.wlo:
                itab[:, g.ioff_lo: g.ioff_lo + 8 * g.tiles * g.wlo] = _wrap_idx16(flat_lo)
            if g.whi:
                itab[:, g.ioff_hi: g.ioff_hi + 8 * g.tiles * g.whi] = _wrap_idx16(flat_hi)
        idx_tabs.append(itab)
        mask_tabs.append(mtab)

    lay = Layout()
    lay.N, lay.R, lay.T, lay.chunk = N, R, T, chunk
    lay.tiles_per_ag, lay.tile_start, lay.CR, lay.agbase = tiles_per_ag, tile_start, CR, agbase
    lay.LO_ROWS, lay.TOT_ROWS = LO_ROWS, TOT_ROWS
    lay.perm, lay.row_of = perm, row_of_arr
    lay.groups, lay.w0_runs = groups, w0_runs
    lay.IC, lay.MC = IC, MC
    lay.idx_tabs, lay.mask_tabs = idx_tabs, mask_tabs
    lay.max_cols = max(g.tiles * (g.wlo + g.whi) for g in groups)
    lay.max_half = max(max(g.tiles * g.wlo, g.tiles * g.whi) for g in groups)
    lay.max_tiles = max(g.tiles for g in groups)
    return lay


def host_inputs(lay, x, qkv_w, qkv_b, out_w, out_b, out_w_last, out_b_last):
    x = np.asarray(x, dtype=np.float32)
    nclass = out_w_last.shape[1]
    wc = np.zeros((L, D, 3 * D), dtype=np.float16)
    bc = np.zeros((L, P, 3 * D), dtype=np.float16)
    wo = np.zeros((L, D, D), dtype=np.float16)
    bo = np.zeros((L, P, D), dtype=np.float16)
    for l in range(L):
        wq, wk, wv = qkv_w[l, 0], qkv_w[l, 1], qkv_w[l, 2]
        bq, bk, bv = qkv_b[l, 0], qkv_b[l, 1], qkv_b[l, 2]
        wc[l] = np.concatenate(
            [wq * SQ[l], wk * (SCALE / SQ[l]), wv * SV[l]], axis=1
        ).astype(np.float16)
        bcl = np.concatenate(
            [bq * SQ[l], bk * (SCALE / SQ[l]), bv * SV[l]]
        ).astype(np.float16)
        bc[l] = np.tile(bcl[None, :], (P, 1))
        if l < L - 1:
            wo[l] = (out_w[l] / SV[l]).astype(np.float16)
            bo[l] = np.tile(out_b[l][None, :].astype(np.float16), (P, 1))
        else:
            wo[l, :, :nclass] = (out_w_last / SV[l]).astype(np.float16)
            bo[l, :, :nclass] = np.tile(out_b_last[None, :].astype(np.float16), (P, 1))

    in_maps = []
    for c in range(NCORES):
        m = {
            "x0": np.where(
                (lay.perm[c] >= 0)[:, None], x[np.maximum(lay.perm[c], 0)], 0.0
            ).astype(np.float16),
            "wc": wc, "bc": bc, "wo": wo, "bo": bo,
            "idx": lay.idx_tabs[c], "mask": lay.mask_tabs[c],
        }
        in_maps.append(m)
    return in_maps


def host_output(lay, outs, nclass):
    full = np.zeros((lay.N, nclass), dtype=np.float32)
    for c in range(NCORES):
        real = lay.perm[c] >= 0
        full[lay.perm[c][real]] = outs[c][real]
    return full


# ----------------------------------------------------------------------------
# Device program
# ----------------------------------------------------------------------------

def _tree_reduce_w(nc, pool, tmp2_view_fn, Tg, w, tag):
    """Emit a pairwise-add tree over the w axis of [P, Tg, w, D] fp16 data.

    tmp2_view_fn(w0, w1) -> AP [P, Tg, w1-w0, D] view of the source region.
    Returns an AP [P, Tg, 1, D] with the sums (may alias a scratch tile).
    """
    if w == 1:
        return tmp2_view_fn(0, 1)
    cur_view = tmp2_view_fn
    cur_w = w
    ping = 0
    scr = [None, None]
    while cur_w > 1:
        half = cur_w // 2
        ceilh = cur_w - half
        dst_t = pool.tile([P, Tg * ceilh * D], F16, tag=f"{tag}{ping}",
                          name=f"{tag}{ping}")
        dstv = dst_t[:].rearrange("p (t w d) -> p t w d", w=ceilh, d=D)
        nc.vector.tensor_tensor(
            out=dstv[:, :, 0:half, :],
            in0=cur_view(0, half),
            in1=cur_view(ceilh, cur_w),
            op=OP.add,
        )
        if ceilh > half:
            nc.scalar.copy(out=dstv[:, :, half:ceilh, :], in_=cur_view(half, ceilh))
        dv = dstv

        def cur_view(w0, w1, _dv=dv):
            return _dv[:, :, w0:w1, :]

        cur_w = ceilh
        ping ^= 1
    return cur_view(0, 1)


def build_nc(lay, nclass):
    R, T = lay.R, lay.T
    LO_ROWS, TOT_ROWS = lay.LO_ROWS, lay.TOT_ROWS
    nc = bacc.Bacc(trn_type="TRN2", num_devices=NCORES, num_swdge_queues=NQ,
                   dynamic_dma_scratch_size=32768)

    x0 = nc.dram_tensor("x0", [R, D], F16, kind="ExternalInput")
    wc = nc.dram_tensor("wc", [L, D, 3 * D], F16, kind="ExternalInput")
    bc = nc.dram_tensor("bc", [L, P, 3 * D], F16, kind="ExternalInput")
    wo = nc.dram_tensor("wo", [L, D, D], F16, kind="ExternalInput")
    bo = nc.dram_tensor("bo", [L, P, D], F16, kind="ExternalInput")
    idx_d = nc.dram_tensor("idx", [P, lay.IC], I16, kind="ExternalInput")
    mask_d = nc.dram_tensor("mask", [P, lay.MC], F16, kind="ExternalInput")
    out_ext = nc.dram_tensor("out", [R, nclass], FP, kind="ExternalOutput")

    qv_slice = [
        nc.dram_tensor(f"qv_slice{l}", [R, 2 * D], F8, kind="Internal")
        for l in range(L)
    ]
    qv_lo = [
        nc.dram_tensor(f"qv_lo{l}", [LO_ROWS, 2 * D], F8, kind="Internal",
                       addr_space="Shared")
        for l in range(L)
    ]
    qv_hi = [
        nc.dram_tensor(f"qv_hi{l}", [TOT_ROWS - LO_ROWS, 2 * D], F8,
                       kind="Internal", addr_space="Shared")
        for l in range(L)
    ]
    rg = [list(range(NCORES))]

    with tile.TileContext(nc) as tc:
        with (
            tc.tile_pool(name="const", bufs=1) as cpool,
            tc.tile_pool(name="persist", bufs=1) as ppool,
            tc.tile_pool(name="proj", bufs=4) as projpool,
            tc.tile_pool(name="qvl", bufs=4) as qvlpool,
            tc.tile_pool(name="qvh", bufs=2) as qvhpool,
            tc.tile_pool(name="tmp", bufs=2) as tpool,
            tc.tile_pool(name="tree", bufs=2) as trpool,
            tc.tile_pool(name="scr", bufs=2) as spool,
            tc.tile_pool(name="sm", bufs=2) as smpool,
            tc.tile_pool(name="agg", bufs=2) as apool,
            tc.tile_pool(name="psA", bufs=2, space="PSUM") as psA,
            tc.tile_pool(name="psB", bufs=2, space="PSUM") as psB,
            tc.tile_pool(name="psC", bufs=2, space="PSUM") as psC,
        ):
            dma_sems = [nc.alloc_semaphore(f"swdge_dma{q}") for q in range(NQ)]
            for q in range(NQ):
                nc.gpsimd.sem_clear(dma_sems[q])
            qnext = [0]   # round-robin SWDGE queue cursor
            cum16 = [0] * NQ  # completion-sem target per queue (16 per DMA)
            last_q = [None] * NQ  # last prep/trigger name per queue

            def _chain_q(inst, q):
                """Serialize each queue's prep/trigger stream with no-sync
                deps: trigger_n(count) fires the count OLDEST ring entries,
                so Pool must execute a queue's preps in emission order."""
                if last_q[q] is not None:
                    deps = InstructionNameOrderedSet()
                    deps.add(last_q[q])
                    inst.ins.add_nosync_dependencies_from(deps)
                last_q[q] = inst.ins.name
                return inst

            ident = cpool.tile([P, P], F16, tag="ident", name="ident")
            make_identity(nc, ident[:])
            wc_sb = cpool.tile([P, L * 3 * D], F16, tag="wc", name="wc")
            nc.sync.dma_start(wc_sb[:].rearrange("k (l n) -> k l n", l=L),
                              wc[:].rearrange("l k n -> k l n"))
            bc_sb = cpool.tile([P, L * 3 * D], F16, tag="bc", name="bc")
            nc.sync.dma_start(bc_sb[:].rearrange("p (l n) -> p l n", l=L),
                              bc[:].rearrange("l p n -> p l n"))
            wo_sb = cpool.tile([P, L * D], F16, tag="wo", name="wo")
            nc.sync.dma_start(wo_sb[:].rearrange("k (l n) -> k l n", l=L),
                              wo[:].rearrange("l k n -> k l n"))
            bo_sb = cpool.tile([P, L * D], F16, tag="bo", name="bo")
            nc.sync.dma_start(bo_sb[:].rearrange("p (l n) -> p l n", l=L),
                              bo[:].rearrange("l p n -> p l n"))
            idx_sb = cpool.tile([P, lay.IC], I16, tag="idx", name="idx")
            nc.sync.dma_start(idx_sb[:], idx_d[:])
            mask_sb = cpool.tile([P, lay.MC], F16, tag="mask", name="mask")
            nc.sync.dma_start(mask_sb[:], mask_d[:])

            x_sb = ppool.tile([P, T * D], F16, tag="x", name="x")
            nc.sync.dma_start(x_sb[:].rearrange("p (t f) -> p t f", f=D),
                              x0[:].rearrange("(t p) f -> p t f", p=P))
            k_sb = ppool.tile([P, T * D], F16, tag="k", name="k")
            tc.strict_bb_all_engine_barrier()

            proj_pend = []  # software skew: transpose ahead of matmul

            def _proj_stage_a(l, t):
                xT_ps = psA.tile([P, P], F16, tag="xT", name="xT")
                nc.tensor.transpose(
                    xT_ps[:], x_sb[:, t * D:(t + 1) * D], ident[:]
                )
                xT = projpool.tile([P, P], F16, tag="xT_sb", name="xT_sb")
                nc.scalar.copy(out=xT[:], in_=xT_ps[:])
                return xT

            def _proj_stage_b(l, t, xT):
                qkv_ps = psB.tile([P, 3 * D], FP, tag="qkv", name="qkv")
                nc.tensor.matmul(
                    qkv_ps[:], lhsT=xT[:],
                    rhs=wc_sb[:, l * 3 * D:(l + 1) * 3 * D],
                    start=True, stop=True,
                )
                qv_st = projpool.tile([P, 2 * D], F16, tag="qv_st", name="qv_st")
                bofs = l * 3 * D
                nc.vector.tensor_tensor(
                    out=qv_st[:, 0:D], in0=qkv_ps[:, 0:D],
                    in1=bc_sb[:, bofs:bofs + D], op=OP.add,
                )
                nc.vector.tensor_tensor(
                    out=qv_st[:, D:2 * D], in0=qkv_ps[:, 2 * D:3 * D],
                    in1=bc_sb[:, bofs + 2 * D:bofs + 3 * D], op=OP.add,
                )
                nc.vector.tensor_tensor(
                    out=k_sb[:, t * D:(t + 1) * D], in0=qkv_ps[:, D:2 * D],
                    in1=bc_sb[:, bofs + D:bofs + 2 * D], op=OP.add,
                )
                # fp32->fp8 does NOT saturate (overflow -> inf): clamp first
                qv8 = projpool.tile([P, 2 * D], F8, tag="qv8", name="qv8")
                with nc.allow_low_precision("fp8 qv table; tol 2e-2"):
                    nc.vector.tensor_scalar(
                        out=qv8[:], in0=qv_st[:], scalar1=F8_MAX,
                        scalar2=-F8_MAX, op0=OP.min, op1=OP.max,
                    )
                nc.sync.dma_start(qv_slice[l][t * P:(t + 1) * P, :], qv8[:])

            def emit_proj_tile(l, t):
                proj_pend.append((l, t, _proj_stage_a(l, t)))
                if len(proj_pend) > 1:
                    ll, tt, xT = proj_pend.pop(0)
                    _proj_stage_b(ll, tt, xT)

            def flush_proj():
                while proj_pend:
                    ll, tt, xT = proj_pend.pop(0)
                    _proj_stage_b(ll, tt, xT)

            def emit_ag(l, j):
                flush_proj()
                r0 = lay.tile_start[j] * P
                r1 = lay.tile_start[j + 1] * P
                g0 = int(lay.agbase[j])
                g1 = int(lay.agbase[j + 1])
                dst = (qv_lo[l][g0:g1, :] if g1 <= LO_ROWS
                       else qv_hi[l][g0 - LO_ROWS:g1 - LO_ROWS, :])
                nc.gpsimd.collective_compute(
                    "AllGather", OP.bypass, replica_groups=rg,
                    ins=[qv_slice[l][r0:r1, :]], outs=[dst],
                )

            def emit_w0(l, t0, tn):
                if l < L - 1:
                    nc.scalar.copy(
                        out=x_sb[:, t0 * D:(t0 + tn) * D].rearrange(
                            "p (t d) -> p t d", d=D),
                        in_=bo_sb[:, l * D:(l + 1) * D].unsqueeze(1)
                            .to_broadcast([P, tn, D]),
                    )
                else:
                    o_sb = projpool.tile([P, tn * nclass], FP, tag="o_sb0",
                                         name="o_sb0")
                    nc.scalar.copy(
                        out=o_sb[:].rearrange("p (t d) -> p t d", d=nclass),
                        in_=bo_sb[:, l * D:l * D + nclass].unsqueeze(1)
                            .to_broadcast([P, tn, nclass]),
                    )
                    nc.sync.dma_start(
                        out_ext[t0 * P:(t0 + tn) * P, :].rearrange(
                            "(t p) d -> p t d", p=P),
                        o_sb[:].rearrange("p (t d) -> p t d", d=nclass),
                    )

            # One SWDGE queue per group-half.  Tile's own DMA-completion
            # tracking is unsound here (its DMASW lane sems assume
            # single-queue FIFO completion, and gen_mode==1 preps tick on
            # desc-gen), so consumers carry explicit wait_op()s on the
            # per-queue DMA semaphore instead.
            GCH = 64  # slot-columns per gather (64*128 = 8192 idxs)

            def emit_half_gathers(l, g, hi):
                """Prep+trigger one half's gathers -> (tile, queue, sem_tgt).

                lo halves need only AG chunks 0-1, so the layer loop emits
                them a few groups ahead to keep Q7 desc-gen busy while the
                hi chunks' AllGather lands.
                """
                w = g.whi if hi else g.wlo
                cols_h = g.tiles * w
                if not cols_h:
                    return None
                pool, tag = (qvhpool, "qvgh") if hi else (qvlpool, "qvgl")
                gt = pool.tile([P, cols_h * 2 * D], F8, tag=tag, name=tag)
                q = qnext[0]
                qnext[0] = (q + 1) % NQ
                ioff = g.ioff_hi if hi else g.ioff_lo
                in_ap = (qv_hi[l][:, :] if hi else qv_lo[l][:, :])
                for cc in range(0, cols_h, GCH):
                    cn = min(GCH, cols_h - cc)
                    _chain_q(nc.gpsimd.dma_gather(
                        out_ap=gt[:, cc * 2 * D:(cc + cn) * 2 * D].rearrange(
                            "p (c e) -> p c e", e=2 * D),
                        in_ap=in_ap,
                        idxs_ap=idx_sb[:, ioff + 8 * cc:
                                       ioff + 8 * (cc + cn)],
                        num_idxs=P * cn, num_idxs_reg=P * cn,
                        elem_size=2 * D, queue_num=q,
                        prepare_only=True, sem=dma_sems[q],
                        single_packet=False,
                    ), q)
                    cum16[q] += 16
                _chain_q(nc.gpsimd.trigger_dma(count=None, queue_num=q), q)
                return (gt, q, cum16[q])

            def emit_group_compute(l, g, glo, ghi):
                Tg, wlo, whi = g.tiles, g.wlo, g.whi
                W = wlo + whi
                cols = Tg * W
                colsLo, colsHi = Tg * wlo, Tg * whi
                nt0 = g.nt0
                kv = k_sb[:, nt0 * D:(nt0 + Tg) * D].rearrange(
                    "p (t d) -> p t d", d=D)

                # scores: tmp = q * k (broadcast over slots); DVE reads the
                # gathered fp8 directly (1x mode; DVE has slack under the Q7
                # wall) and each first consumer carries the explicit
                # DMA-completion wait for its half
                tmp = tpool.tile([P, cols * D], F16, tag="tmp", name="tmp")
                if wlo:
                    gt, q, sem_tgt = glo
                    nc.vector.tensor_tensor(
                        out=tmp[:, :colsLo * D].rearrange(
                            "p (t w d) -> p t w d", w=wlo, d=D),
                        in0=gt[:].rearrange(
                            "p (t w e) -> p t w e", w=wlo, e=2 * D)[:, :, :, 0:D],
                        in1=kv.unsqueeze(2).to_broadcast([P, Tg, wlo, D]),
                        op=OP.mult,
                    ).wait_op(dma_sems[q], sem_tgt, "sem-ge")
                if whi:
                    gt, q, sem_tgt = ghi
                    nc.vector.tensor_tensor(
                        out=tmp[:, colsLo * D:].rearrange(
                            "p (t w d) -> p t w d", w=whi, d=D),
                        in0=gt[:].rearrange(
                            "p (t w e) -> p t w e", w=whi, e=2 * D)[:, :, :, 0:D],
                        in1=kv.unsqueeze(2).to_broadcast([P, Tg, whi, D]),
                        op=OP.mult,
                    ).wait_op(dma_sems[q], sem_tgt, "sem-ge")

                # single reduce over d per (slot, head); fp16 out
                s_t = smpool.tile([P, cols * H], F16, tag="s", name="s")
                with nc.allow_low_precision("fp16 score sum; tol 2e-2"):
                    nc.vector.reduce_sum(
                        s_t[:],
                        tmp[:].rearrange("p (c h d) -> p c h d", h=H, d=HD),
                        axis=AX.X)

                # scores are small (|s| < 3): skip the max-subtract shift,
                # exponentiate directly (fp16-safe; pads masked after)
                ex = smpool.tile([P, cols * H], F16, tag="ex", name="ex")
                nc.scalar.activation(
                    out=ex[:], in_=s_t[:],
                    func=mybir.ActivationFunctionType.Exp)
                # mask: mask table is laid (t, w) with w in [0, W) combined
                for (c0, w0, wr) in ((0, 0, wlo), (colsLo, wlo, whi)):
                    if not wr:
                        continue
                    exv = ex[:, c0 * H:(c0 + Tg * wr) * H].rearrange(
                        "p (t w h) -> p t w h", w=wr, h=H)
                    mv = mask_sb[:, g.moff:g.moff + cols].rearrange(
                        "p (t w) -> p t w", w=W)[:, :, w0:w0 + wr]
                    nc.vector.tensor_tensor(
                        out=exv, in0=exv,
                        in1=mv.unsqueeze(3).to_broadcast([P, Tg, wr, H]),
                        op=OP.mult)

                # denom (fp32) + eps, rec = 1/denom
                ex_lo = ex[:, :colsLo * H].rearrange(
                    "p (t w h) -> p t h w", w=max(wlo, 1), h=H)
                ex_hi = ex[:, colsLo * H:].rearrange(
                    "p (t w h) -> p t h w", w=max(whi, 1), h=H)
                den = smpool.tile([P, Tg * H], FP, tag="den", name="den")
                if wlo and whi:
                    den2 = smpool.tile([P, Tg * H], FP, tag="den2", name="den2")
                    nc.vector.reduce_sum(den[:], ex_lo, axis=AX.X)
                    nc.vector.reduce_sum(den2[:], ex_hi, axis=AX.X)
                    nc.vector.scalar_tensor_tensor(
                        out=den[:], in0=den[:], scalar=DEN_EPS, in1=den2[:],
                        op0=OP.add, op1=OP.add)
                elif wlo:
                    nc.vector.reduce_sum(den[:], ex_lo, axis=AX.X)
                    nc.vector.tensor_scalar_add(out=den[:], in0=den[:],
                                                scalar1=DEN_EPS)
                else:
                    nc.vector.reduce_sum(den[:], ex_hi, axis=AX.X)
                    nc.vector.tensor_scalar_add(out=den[:], in0=den[:],
                                                scalar1=DEN_EPS)
                rec = smpool.tile([P, Tg * H], FP, tag="rec", name="rec")
                nc.vector.reciprocal(rec[:], den[:])

                # tmp2 = v * ex   (3 free dims: (t w) merged per half)
                tmp2 = tpool.tile([P, cols * D], F16, tag="tmp2", name="tmp2")
                for hi, (c0, wr, gh) in enumerate(
                        ((0, wlo, glo), (colsLo, whi, ghi))):
                    if not wr:
                        continue
                    ncols = Tg * wr
                    gt, q, sem_tgt = gh
                    nc.vector.tensor_tensor(
                        out=tmp2[:, c0 * D:(c0 + ncols) * D].rearrange(
                            "p (c h d) -> p c h d", h=H, d=HD),
                        in0=gt[:].rearrange(
                            "p (c e) -> p c e", e=2 * D)[:, :, D:2 * D].rearrange(
                            "p c (h d) -> p c h d", d=HD),
                        in1=ex[:, c0 * H:(c0 + ncols) * H].rearrange(
                            "p (c h) -> p c h", h=H).unsqueeze(3)
                            .to_broadcast([P, ncols, H, HD]),
                        op=OP.mult).wait_op(dma_sems[q], sem_tgt, "sem-ge")

                # aggregate over w (pairwise tree), merge halves, normalize
                def lo_view(w0, w1):
                    return tmp2[:, :colsLo * D].rearrange(
                        "p (t w d) -> p t w d", w=wlo, d=D)[:, :, w0:w1, :]

                def hi_view(w0, w1):
                    return tmp2[:, colsLo * D:].rearrange(
                        "p (t w d) -> p t w d", w=whi, d=D)[:, :, w0:w1, :]

                aggN = apool.tile([P, Tg * D], F16, tag="aggN", name="aggN")
                aggU = apool.tile([P, Tg * D], F16, tag="aggU", name="aggU")
                aggUv = aggU[:].rearrange("p (t d) -> p t d", d=D).unsqueeze(2)
                if wlo and whi:
                    alo = _tree_reduce_w(nc, trpool, lo_view, Tg, wlo, "g")
                    nc.scalar.copy(out=aggUv, in_=alo)
                    ahi = _tree_reduce_w(nc, trpool, hi_view, Tg, whi, "g")
                    nc.vector.tensor_tensor(out=aggUv, in0=aggUv, in1=ahi,
                                            op=OP.add)
                else:
                    view = lo_view if wlo else hi_view
                    ww = wlo if wlo else whi
                    af = _tree_reduce_w(nc, trpool, view, Tg, ww, "g")
                    nc.scalar.copy(out=aggUv, in_=af)
                nc.vector.tensor_tensor(
                    out=aggN[:].rearrange("p (t h d) -> p t h d", h=H, d=HD),
                    in0=aggU[:].rearrange("p (t h d) -> p t h d", h=H, d=HD),
                    in1=rec[:].rearrange("p (t h) -> p t h", h=H).unsqueeze(3)
                        .to_broadcast([P, Tg, H, HD]),
                    op=OP.mult)

                # fused output projection per tile
                for ti in range(Tg):
                    nt = nt0 + ti
                    aT_ps = psA.tile([P, P], F16, tag="aT", name="aT")
                    nc.tensor.transpose(
                        aT_ps[:], aggN[:, ti * D:(ti + 1) * D], ident[:])
                    aT = projpool.tile([P, P], F16, tag="aT_sb", name="aT_sb")
                    nc.scalar.copy(out=aT[:], in_=aT_ps[:])
                    o_ps = psC.tile([P, D], FP, tag="o", name="o")
                    nc.tensor.matmul(
                        o_ps[:], lhsT=aT[:], rhs=wo_sb[:, l * D:(l + 1) * D],
                        start=True, stop=True,
                    )
                    if l < L - 1:
                        nc.vector.tensor_tensor(
                            out=x_sb[:, nt * D:(nt + 1) * D], in0=o_ps[:],
                            in1=bo_sb[:, l * D:(l + 1) * D], op=OP.add,
                        )
                    else:
                        o_sb = projpool.tile([P, nclass], FP, tag="o_sb",
                                             name="o_sb")
                        nc.vector.tensor_tensor(
                            out=o_sb[:], in0=o_ps[:, 0:nclass],
                            in1=bo_sb[:, l * D:l * D + nclass], op=OP.add,
                        )
                        nc.sync.dma_start(
                            out_ext[nt * P:(nt + 1) * P, :], o_sb[:])

            # edge-phase segments in tile order: w0 runs + gather groups
            segments = sorted(
                [("w0", t0, tn) for (t0, tn) in lay.w0_runs]
                + [("g", g.nt0, g) for g in lay.groups],
                key=lambda x: x[1])

            # layer-0 prologue: project + AllGather (nothing to overlap with)
            for j in range(NCH):
                for t in range(lay.tile_start[j], lay.tile_start[j + 1]):
                    emit_proj_tile(0, t)
                emit_ag(0, j)

            glist = [s[2] for s in segments if s[0] == "g"]
            KLO = 2  # lo-gather lookahead depth (groups; ~2.5x bigger at cap 64)

            for l in range(L):
                nxt = l + 1
                next_chunk = 0
                projected = 0
                gj = 0
                pend_lo = {}
                if not SKIP_EDGE:
                    # lo-half gathers depend only on AG chunks 0-1: run a few
                    # ahead so Q7 stays busy while hi chunks' AG finishes
                    for j in range(min(KLO, len(glist))):
                        pend_lo[j] = emit_half_gathers(l, glist[j], hi=False)
                for seg in segments:
                    if seg[0] == "w0":
                        emit_w0(l, seg[1], seg[2])
                        done = seg[1] + seg[2]
                    else:
                        g = seg[2]
                        if not SKIP_EDGE:
                            if gj + KLO < len(glist):
                                pend_lo[gj + KLO] = emit_half_gathers(
                                    l, glist[gj + KLO], hi=False)
                            ghi = emit_half_gathers(l, g, hi=True)
                            glo = pend_lo.pop(gj)
                            if not GATHER_ONLY:
                                emit_group_compute(l, g, glo, ghi)
                        gj += 1
                        done = g.nt0 + g.tiles
                    # software-pipeline: project layer l+1 tiles whose edge
                    # output is complete; fire its AllGather chunks eagerly
                    while (nxt < L and next_chunk < NCH
                           and lay.tile_start[next_chunk + 1] <= done):
                        for t in range(projected,
                                       lay.tile_start[next_chunk + 1]):
                            emit_proj_tile(nxt, t)
                        projected = lay.tile_start[next_chunk + 1]
                        emit_ag(nxt, next_chunk)
                        next_chunk += 1
                while nxt < L and next_chunk < NCH:
                    for t in range(projected, lay.tile_start[next_chunk + 1]):
                        emit_proj_tile(nxt, t)
                    projected = lay.tile_start[next_chunk + 1]
                    emit_ag(nxt, next_chunk)
                    next_chunk += 1
    nc.compile()
    return nc


# ----------------------------------------------------------------------------
# Entry point
# ----------------------------------------------------------------------------

_trace = [False]  # test.py can flip this to profile


def kernel(x, src, dst, qkv_w, qkv_b, out_w, out_b, out_w_last, out_b_last):
    x = np.asarray(x, dtype=np.float32)
    lay = build_layout(np.asarray(src), np.asarray(dst), x.shape[0])
    nclass = np.asarray(out_w_last).shape[1]
    in_maps = host_inputs(
        lay, x, np.asarray(qkv_w, dtype=np.float32),
        np.asarray(qkv_b, dtype=np.float32), np.asarray(out_w, dtype=np.float32),
        np.asarray(out_b, dtype=np.float32), np.asarray(out_w_last, dtype=np.float32),
        np.asarray(out_b_last, dtype=np.float32),
    )
    nc = build_nc(lay, nclass)
    res = run_bass_kernel_spmd(
        nc, in_maps, core_ids=list(range(NCORES)), trace=_trace[0]
    )
    kernel.last_results = res
    outs = [res.results[c]["out"] for c in range(NCORES)]
    return host_output(lay, outs, nclass)



# revision 43
# speedup vs baseline: 1.2450x; 1.2450x over previous
"""Graph-Transformer message-passing kernel for 8 Trainium2 NeuronCores.

Strategy (1D dst-shard, fp8e3 qv table, pipelined SWDGE gathers):
  - Dst nodes are DEALT to the 8 cores by degree rank so all cores see
    near-identical degree profiles; each core owns all in-edges of its nodes,
    so segment softmax/aggregation are fully local.  Per-tile edge-slot
    widths are the 8-core max, kept tight by the dealing.
  - int16 gather indices force a lo/hi source-table split (<=32768 rows
    each); the lo source SET is chosen by greedy discrepancy minimization so
    every dst's in-degree splits ~50/50, collapsing the binomial width
    padding (~13% fewer gather slots).
  - Per layer, each core projects Q|K|V (PE), writes q|v scaled+clamped to
    fp8e3 (per-layer scales folded into host-side weights), AllGathers the
    table in 6 chunks overlapped with projection, then bulk-gathers source
    rows with dma_gather in prepare_only mode: Q7 desc-gen (the kernel's
    bottleneck, ~12ns/edge) retires without blocking on the SDMA drain, 4
    SWDGE queues round-robin, and explicit per-queue DMA-completion
    semaphores guard the consumers (Tile's own DMASW tracking is unsound for
    multi-queue/prep gathers).  lo-half gathers run a few groups ahead so
    desc-gen covers the hi chunks' AllGather tail.
  - Gathered fp8 rows are upconverted on ACT; masked edge softmax and the
    weighted aggregation run on DVE with tree reductions; output projection
    on PE.  3 layers fuse into one NEFF; the host inverts the permutation.
"""

import os
import numpy as np

import concourse.bass as bass
import concourse.bacc as bacc
import concourse.mybir as mybir
import concourse.tile as tile
from concourse.instruction_name_ordered_set import InstructionNameOrderedSet
from concourse.masks import make_identity
from concourse.bass_utils import run_bass_kernel_spmd

NCORES = 8
L = 3
H = 8
D = 128
HD = D // H
SCALE = 1.0 / float(np.sqrt(HD))
P = 128
NCH = 6              # AllGather chunks per layer
GROUP_COLS_CAP = 32  # max slot-columns (tiles*width) per gather group
GROUP_WASTE_CAP = 0.06
DEN_EPS = 6e-5
NQ = 4               # SWDGE queues for gather desc-gen / drain pipelining
ONE_AG = bool(int(os.environ.get("K_ONE_AG", "0")))
SKIP_EDGE = bool(int(os.environ.get("K_SKIP_EDGE", "0")))
GATHER_ONLY = bool(int(os.environ.get("K_GATHER_ONLY", "0")))

FP = mybir.dt.float32
F16 = mybir.dt.float16
F8 = mybir.dt.float8e3
I16 = mybir.dt.int16
AX = mybir.AxisListType
OP = mybir.AluOpType

# The q|v table is stored fp8e3 (1.9% RMS quantization) to halve the
# AllGather and gather-drain bytes.  Per-layer scales keep values in
# e3m4's [0.25, 15.5] sweet spot; they fold into the host-side weights
# (q: into wq/bq, undone via wk; v: into wv/bv, undone via wo).
F8_MAX = 15.5
SQ = [4.0, 16.0, 32.0]
SV = [4.0, 16.0, 32.0]


# ----------------------------------------------------------------------------
# Host-side layout
# ----------------------------------------------------------------------------

class Layout:
    pass


class Group:
    pass


def _wrap_idx16(flat):
    """int16 idx list -> [128, n/16] wrapped layout (16-partition, replicated)."""
    n = len(flat)
    assert n % 16 == 0
    arr = np.asarray(flat, dtype=np.int16).reshape(n // 16, 16).T  # [16, n/16]
    return np.tile(arr, (8, 1))  # [128, n/16]


def build_layout(src, dst, n_nodes):
    src = np.asarray(src).astype(np.int64)
    dst = np.asarray(dst).astype(np.int64)
    N = n_nodes
    chunk = (N + NCORES - 1) // NCORES
    T = (chunk + P - 1) // P
    R = T * P

    # AllGather tile chunks; lo/hi row split is at the boundary after chunk 1.
    # Within each half, the last chunk is kept small so the serial tail
    # (last out-proj -> proj -> AG -> first gather) is short.
    lo_tiles_n = (T + 1) // 2
    hi_tiles_n = T - lo_tiles_n
    def _split_lo(n):
        return [(n + 1) // 2, n - (n + 1) // 2]
    def _split_hi(n):
        # decreasing sizes: the last chunk gates the next layer's gathers
        a = max(1, (n * 4) // 10)
        b = max(1, (n * 3) // 10)
        c = max(1, (n * 2) // 10)
        return [a, b, c, n - a - b - c]
    tiles_per_ag = _split_lo(lo_tiles_n) + _split_hi(hi_tiles_n)
    tile_start = np.cumsum([0] + tiles_per_ag)  # len NCH+1
    CR = [tiles_per_ag[j] * P for j in range(NCH)]
    agbase = np.cumsum([0] + [NCORES * c for c in CR])  # global row base per chunk
    t_split = tile_start[2]          # tiles in the lo half (first two chunks)
    LO_POS = t_split * P             # per-core positions in lo half
    LO_ROWS = int(agbase[2])         # global rows in lo half
    TOT_ROWS = int(agbase[NCH])
    assert LO_ROWS <= 32768 and TOT_ROWS - LO_ROWS <= 32768

    def grow_of(core, pos):
        t = pos // P
        j = int(np.searchsorted(tile_start, t, side="right")) - 1
        return int(agbase[j]) + core * CR[j] + (pos - tile_start[j] * P)

    # --- choose the lo source set to BALANCE every dst's lo/hi in-degree ---
    # The int16 gather indices force a 2-table (lo/hi) source split; a random
    # split costs binomial variance in the per-tile widths.  Greedy
    # discrepancy minimization keeps dlo ~= dhi per dst, collapsing that
    # variance (and with it ~13% of the gather slots = Q7 desc-gen time).
    NLO = LO_POS * NCORES
    e_order = np.argsort(src, kind="stable")
    dst_by_src = dst[e_order]
    odeg = np.bincount(src, minlength=N)
    ostart = np.zeros(N + 1, dtype=np.int64)
    np.cumsum(odeg, out=ostart[1:])
    pot = np.zeros(N, dtype=np.int64)   # 2*dlo_assigned - deg_assigned
    is_lo = np.zeros(N, dtype=bool)
    quota, remaining = NLO, N
    for s in np.argsort(-odeg, kind="stable"):
        ds = dst_by_src[ostart[s]:ostart[s + 1]]
        p = pot[ds]
        go_lo = np.abs(p + 1).sum() < np.abs(p - 1).sum()
        if quota == 0:
            go_lo = False
        elif quota == remaining:
            go_lo = True
        if go_lo:
            is_lo[s] = True
            pot[ds] += 1
            quota -= 1
        else:
            pot[ds] -= 1
        remaining -= 1
    assert int(is_lo.sum()) == NLO
    edge_is_lo = is_lo[src]

    dlo = np.bincount(dst[edge_is_lo], minlength=N)
    dhi = np.bincount(dst[~edge_is_lo], minlength=N)

    # --- deal nodes to cores by degree rank (all cores see near-identical
    # degree profiles, so the 8-core per-tile width max stays tight);
    # auto-tune the sort key for minimal padding ---
    los = np.nonzero(is_lo)[0]
    his = np.nonzero(~is_lo)[0]
    keysets = [
        lambda g: np.lexsort((dhi[g], np.maximum(dlo[g], dhi[g]))),
        lambda g: np.lexsort((dlo[g], np.maximum(dlo[g], dhi[g]))),
        lambda g: np.lexsort((np.minimum(dlo[g], dhi[g]),
                              np.maximum(dlo[g], dhi[g]))),
        lambda g: np.lexsort((dhi[g] - dlo[g], dlo[g] + dhi[g])),
    ]
    best = None
    for key in keysets:
        perm_k = np.full((NCORES, R), -1, dtype=np.int64)
        lo_sorted = los[key(los)]
        hi_sorted = his[key(his)]
        for c in range(NCORES):
            sel = lo_sorted[c::NCORES]
            perm_k[c, : len(sel)] = sel
            # hi half descending: the layer's last group stays light, which
            # shortens the serial tail into the next layer's AllGather
            selh = hi_sorted[c::NCORES][::-1]
            perm_k[c, LO_POS: LO_POS + len(selh)] = selh
        Tn = R // P
        cost = 0
        for t in range(Tn):
            seg = perm_k[:, t * P:(t + 1) * P].reshape(-1)
            seg = seg[seg >= 0]
            if len(seg):
                cost += dlo[seg].max() + dhi[seg].max()
        if best is None or cost < best[0]:
            best = (cost, perm_k)
    perm = best[1]

    row_of = np.full(N, -1, dtype=np.int64)
    pos_of = np.full(N, -1, dtype=np.int64)
    for c in range(NCORES):
        real = perm[c] >= 0
        pp = np.nonzero(real)[0]
        nodes = perm[c][pp]
        pos_of[nodes] = pp
        row_of[nodes] = [grow_of(c, int(p)) for p in pp]
    row_of_arr = row_of

    # --- per-tile widths (max over cores) ---
    wlo_t = np.zeros(T, dtype=np.int64)
    whi_t = np.zeros(T, dtype=np.int64)
    for c in range(NCORES):
        for t in range(T):
            seg = perm[c, t * P:(t + 1) * P]
            seg = seg[seg >= 0]
            if len(seg):
                wlo_t[t] = max(wlo_t[t], dlo[seg].max())
                whi_t[t] = max(whi_t[t], dhi[seg].max())

    # --- group consecutive non-empty tiles ---
    groups = []
    w0_runs = []  # (nt0, ntiles) with W == 0
    t = 0
    gidx = 0
    while t < T:
        if wlo_t[t] + whi_t[t] == 0:
            t0 = t
            while t < T and wlo_t[t] + whi_t[t] == 0:
                t += 1
            w0_runs.append((t0, t - t0))
            continue
        g = Group()
        g.nt0 = t
        g.wlo = int(wlo_t[t])
        g.whi = int(whi_t[t])
        g.tiles = 1
        exact = g.wlo + g.whi
        t += 1
        while t < T and wlo_t[t] + whi_t[t] > 0:
            nw_lo = max(g.wlo, int(wlo_t[t]))
            nw_hi = max(g.whi, int(whi_t[t]))
            ncols = (g.tiles + 1) * (nw_lo + nw_hi)
            nexact = exact + int(wlo_t[t] + whi_t[t])
            if ncols > GROUP_COLS_CAP or (ncols - nexact) > GROUP_WASTE_CAP * nexact:
                break
            g.wlo, g.whi, g.tiles, exact = nw_lo, nw_hi, g.tiles + 1, nexact
            t += 1
        g.idx = gidx
        gidx += 1
        groups.append(g)

    # --- per-core idx / mask tables ---
    order = np.argsort(dst, kind="stable")
    src_sorted = src[order]
    deg = np.bincount(dst, minlength=N)
    starts = np.zeros(N + 1, dtype=np.int64)
    np.cumsum(deg, out=starts[1:])

    idx_cols = []
    mask_cols = 0
    for g in groups:
        g.ioff_lo = sum(idx_cols)
        idx_cols.append(8 * g.tiles * g.wlo)
        g.ioff_hi = sum(idx_cols)
        idx_cols.append(8 * g.tiles * g.whi)
        g.moff = mask_cols
        mask_cols += g.tiles * (g.wlo + g.whi)
    IC = max(sum(idx_cols), 16)
    MC = max(mask_cols, 1)

    idx_tabs = []
    mask_tabs = []
    for c in range(NCORES):
        itab = np.zeros((P, IC), dtype=np.int16)
        mtab = np.zeros((P, MC), dtype=np.float16)
        for g in groups:
            W = g.wlo + g.whi
            flat_lo = np.zeros(g.tiles * g.wlo * P, dtype=np.int16)
            flat_hi = np.zeros(g.tiles * g.whi * P, dtype=np.int16)
            for ti in range(g.tiles):
                tt = g.nt0 + ti
                for pp in range(P):
                    node = perm[c, tt * P + pp]
                    if node < 0:
                        continue
                    es = src_sorted[starts[node]: starts[node] + deg[node]]
                    es_rows = row_of_arr[es]
                    lo_rows = np.sort(es_rows[es_rows < LO_ROWS])
                    hi_rows = np.sort(es_rows[es_rows >= LO_ROWS]) - LO_ROWS
                    assert len(lo_rows) <= g.wlo and len(hi_rows) <= g.whi
                    for w, r in enumerate(lo_rows):
                        flat_lo[(ti * g.wlo + w) * P + pp] = r
                        mtab[pp, g.moff + ti * W + w] = 1.0
                    for w, r in enumerate(hi_rows):
                        flat_hi[(ti * g.whi + w) * P + pp] = r
                        mtab[pp, g.moff + ti * W + g.wlo + w] = 1.0
            if g.wlo:
                itab[:, g.ioff_lo: g.ioff_lo + 8 * g.tiles * g.wlo] = _wrap_idx16(flat_lo)
            if g.whi:
                itab[:, g.ioff_hi: g.ioff_hi + 8 * g.tiles * g.whi] = _wrap_idx16(flat_hi)
        idx_tabs.append(itab)
        mask_tabs.append(mtab)

    lay = Layout()
    lay.N, lay.R, lay.T, lay.chunk = N, R, T, chunk
    lay.tiles_per_ag, lay.tile_start, lay.CR, lay.agbase = tiles_per_ag, tile_start, CR, agbase
    lay.LO_ROWS, lay.TOT_ROWS = LO_ROWS, TOT_ROWS
    lay.perm, lay.row_of = perm, row_of_arr
    lay.groups, lay.w0_runs = groups, w0_runs
    lay.IC, lay.MC = IC, MC
    lay.idx_tabs, lay.mask_tabs = idx_tabs, mask_tabs
    lay.max_cols = max(g.tiles * (g.wlo + g.whi) for g in groups)
    lay.max_half = max(max(g.tiles * g.wlo, g.tiles * g.whi) for g in groups)
    lay.max_tiles = max(g.tiles for g in groups)
    return lay


def host_inputs(lay, x, qkv_w, qkv_b, out_w, out_b, out_w_last, out_b_last):
    x = np.asarray(x, dtype=np.float32)
    nclass = out_w_last.shape[1]
    wc = np.zeros((L, D, 3 * D), dtype=np.float16)
    bc = np.zeros((L, P, 3 * D), dtype=np.float16)
    wo = np.zeros((L, D, D), dtype=np.float16)
    bo = np.zeros((L, P, D), dtype=np.float16)
    for l in range(L):
        wq, wk, wv = qkv_w[l, 0], qkv_w[l, 1], qkv_w[l, 2]
        bq, bk, bv = qkv_b[l, 0], qkv_b[l, 1], qkv_b[l, 2]
        wc[l] = np.concatenate(
            [wq * SQ[l], wk * (SCALE / SQ[l]), wv * SV[l]], axis=1
        ).astype(np.float16)
        bcl = np.concatenate(
            [bq * SQ[l], bk * (SCALE / SQ[l]), bv * SV[l]]
        ).astype(np.float16)
        bc[l] = np.tile(bcl[None, :], (P, 1))
        if l < L - 1:
            wo[l] = (out_w[l] / SV[l]).astype(np.float16)
            bo[l] = np.tile(out_b[l][None, :].astype(np.float16), (P, 1))
        else:
            wo[l, :, :nclass] = (out_w_last / SV[l]).astype(np.float16)
            bo[l, :, :nclass] = np.tile(out_b_last[None, :].astype(np.float16), (P, 1))

    in_maps = []
    for c in range(NCORES):
        m = {
            "x0": np.where(
                (lay.perm[c] >= 0)[:, None], x[np.maximum(lay.perm[c], 0)], 0.0
            ).astype(np.float16),
            "wc": wc, "bc": bc, "wo": wo, "bo": bo,
            "idx": lay.idx_tabs[c], "mask": lay.mask_tabs[c],
        }
        in_maps.append(m)
    return in_maps


def host_output(lay, outs, nclass):
    full = np.zeros((lay.N, nclass), dtype=np.float32)
    for c in range(NCORES):
        real = lay.perm[c] >= 0
        full[lay.perm[c][real]] = outs[c][real]
    return full


# ----------------------------------------------------------------------------
# Device program
# ----------------------------------------------------------------------------

def _tree_reduce_w(nc, pool, tmp2_view_fn, Tg, w, tag):
    """Emit a pairwise-add tree over the w axis of [P, Tg, w, D] fp16 data.

    tmp2_view_fn(w0, w1) -> AP [P, Tg, w1-w0, D] view of the source region.
    Returns an AP [P, Tg, 1, D] with the sums (may alias a scratch tile).
    """
    if w == 1:
        return tmp2_view_fn(0, 1)
    cur_view = tmp2_view_fn
    cur_w = w
    ping = 0
    scr = [None, None]
    while cur_w > 1:
        half = cur_w // 2
        ceilh = cur_w - half
        dst_t = pool.tile([P, Tg * ceilh * D], F16, tag=f"{tag}{ping}",
                          name=f"{tag}{ping}")
        dstv = dst_t[:].rearrange("p (t w d) -> p t w d", w=ceilh, d=D)
        nc.vector.tensor_tensor(
            out=dstv[:, :, 0:half, :],
            in0=cur_view(0, half),
            in1=cur_view(ceilh, cur_w),
            op=OP.add,
        )
        if ceilh > half:
            nc.scalar.copy(out=dstv[:, :, half:ceilh, :], in_=cur_view(half, ceilh))
        dv = dstv

        def cur_view(w0, w1, _dv=dv):
            return _dv[:, :, w0:w1, :]

        cur_w = ceilh
        ping ^= 1
    return cur_view(0, 1)


def build_nc(lay, nclass):
    R, T = lay.R, lay.T
    LO_ROWS, TOT_ROWS = lay.LO_ROWS, lay.TOT_ROWS
    nc = bacc.Bacc(trn_type="TRN2", num_devices=NCORES, num_swdge_queues=NQ,
                   dynamic_dma_scratch_size=32768)

    x0 = nc.dram_tensor("x0", [R, D], F16, kind="ExternalInput")
    wc = nc.dram_tensor("wc", [L, D, 3 * D], F16, kind="ExternalInput")
    bc = nc.dram_tensor("bc", [L, P, 3 * D], F16, kind="ExternalInput")
    wo = nc.dram_tensor("wo", [L, D, D], F16, kind="ExternalInput")
    bo = nc.dram_tensor("bo", [L, P, D], F16, kind="ExternalInput")
    idx_d = nc.dram_tensor("idx", [P, lay.IC], I16, kind="ExternalInput")
    mask_d = nc.dram_tensor("mask", [P, lay.MC], F16, kind="ExternalInput")
    out_ext = nc.dram_tensor("out", [R, nclass], FP, kind="ExternalOutput")

    qv_slice = [
        nc.dram_tensor(f"qv_slice{l}", [R, 2 * D], F8, kind="Internal")
        for l in range(L)
    ]
    qv_lo = [
        nc.dram_tensor(f"qv_lo{l}", [LO_ROWS, 2 * D], F8, kind="Internal",
                       addr_space="Shared")
        for l in range(L)
    ]
    qv_hi = [
        nc.dram_tensor(f"qv_hi{l}", [TOT_ROWS - LO_ROWS, 2 * D], F8,
                       kind="Internal", addr_space="Shared")
        for l in range(L)
    ]
    rg = [list(range(NCORES))]

    with tile.TileContext(nc) as tc:
        with (
            tc.tile_pool(name="const", bufs=1) as cpool,
            tc.tile_pool(name="persist", bufs=1) as ppool,
            tc.tile_pool(name="proj", bufs=4) as projpool,
            tc.tile_pool(name="qvl", bufs=7) as qvlpool,
            tc.tile_pool(name="qvh", bufs=2) as qvhpool,
            tc.tile_pool(name="qv16", bufs=2) as qv16pool,
            tc.tile_pool(name="tmp", bufs=3) as tpool,
            tc.tile_pool(name="tree", bufs=2) as trpool,
            tc.tile_pool(name="scr", bufs=2) as spool,
            tc.tile_pool(name="sm", bufs=2) as smpool,
            tc.tile_pool(name="agg", bufs=2) as apool,
            tc.tile_pool(name="psA", bufs=2, space="PSUM") as psA,
            tc.tile_pool(name="psB", bufs=2, space="PSUM") as psB,
            tc.tile_pool(name="psC", bufs=2, space="PSUM") as psC,
        ):
            dma_sems = [nc.alloc_semaphore(f"swdge_dma{q}") for q in range(NQ)]
            for q in range(NQ):
                nc.gpsimd.sem_clear(dma_sems[q])
            qnext = [0]   # round-robin SWDGE queue cursor
            cum16 = [0] * NQ  # completion-sem target per queue (16 per DMA)
            last_q = [None] * NQ  # last prep/trigger name per queue

            def _chain_q(inst, q):
                """Serialize each queue's prep/trigger stream with no-sync
                deps: trigger_n(count) fires the count OLDEST ring entries,
                so Pool must execute a queue's preps in emission order."""
                if last_q[q] is not None:
                    deps = InstructionNameOrderedSet()
                    deps.add(last_q[q])
                    inst.ins.add_nosync_dependencies_from(deps)
                last_q[q] = inst.ins.name
                return inst

            ident = cpool.tile([P, P], F16, tag="ident", name="ident")
            make_identity(nc, ident[:])
            wc_sb = cpool.tile([P, L * 3 * D], F16, tag="wc", name="wc")
            nc.sync.dma_start(wc_sb[:].rearrange("k (l n) -> k l n", l=L),
                              wc[:].rearrange("l k n -> k l n"))
            bc_sb = cpool.tile([P, L * 3 * D], F16, tag="bc", name="bc")
            nc.sync.dma_start(bc_sb[:].rearrange("p (l n) -> p l n", l=L),
                              bc[:].rearrange("l p n -> p l n"))
            wo_sb = cpool.tile([P, L * D], F16, tag="wo", name="wo")
            nc.sync.dma_start(wo_sb[:].rearrange("k (l n) -> k l n", l=L),
                              wo[:].rearrange("l k n -> k l n"))
            bo_sb = cpool.tile([P, L * D], F16, tag="bo", name="bo")
            nc.sync.dma_start(bo_sb[:].rearrange("p (l n) -> p l n", l=L),
                              bo[:].rearrange("l p n -> p l n"))
            idx_sb = cpool.tile([P, lay.IC], I16, tag="idx", name="idx")
            nc.sync.dma_start(idx_sb[:], idx_d[:])
            mask_sb = cpool.tile([P, lay.MC], F16, tag="mask", name="mask")
            nc.sync.dma_start(mask_sb[:], mask_d[:])

            x_sb = ppool.tile([P, T * D], F16, tag="x", name="x")
            nc.sync.dma_start(x_sb[:].rearrange("p (t f) -> p t f", f=D),
                              x0[:].rearrange("(t p) f -> p t f", p=P))
            k_sb = ppool.tile([P, T * D], F16, tag="k", name="k")
            tc.strict_bb_all_engine_barrier()

            proj_pend = []  # software skew: transpose ahead of matmul

            def _proj_stage_a(l, t):
                xT_ps = psA.tile([P, P], F16, tag="xT", name="xT")
                nc.tensor.transpose(
                    xT_ps[:], x_sb[:, t * D:(t + 1) * D], ident[:]
                )
                xT = projpool.tile([P, P], F16, tag="xT_sb", name="xT_sb")
                nc.scalar.copy(out=xT[:], in_=xT_ps[:])
                return xT

            def _proj_stage_b(l, t, xT):
                qkv_ps = psB.tile([P, 3 * D], FP, tag="qkv", name="qkv")
                nc.tensor.matmul(
                    qkv_ps[:], lhsT=xT[:],
                    rhs=wc_sb[:, l * 3 * D:(l + 1) * 3 * D],
                    start=True, stop=True,
                )
                qv_st = projpool.tile([P, 2 * D], F16, tag="qv_st", name="qv_st")
                bofs = l * 3 * D
                nc.vector.tensor_tensor(
                    out=qv_st[:, 0:D], in0=qkv_ps[:, 0:D],
                    in1=bc_sb[:, bofs:bofs + D], op=OP.add,
                )
                nc.vector.tensor_tensor(
                    out=qv_st[:, D:2 * D], in0=qkv_ps[:, 2 * D:3 * D],
                    in1=bc_sb[:, bofs + 2 * D:bofs + 3 * D], op=OP.add,
                )
                nc.vector.tensor_tensor(
                    out=k_sb[:, t * D:(t + 1) * D], in0=qkv_ps[:, D:2 * D],
                    in1=bc_sb[:, bofs + D:bofs + 2 * D], op=OP.add,
                )
                # fp32->fp8 does NOT saturate (overflow -> inf): clamp first
                qv8 = projpool.tile([P, 2 * D], F8, tag="qv8", name="qv8")
                with nc.allow_low_precision("fp8 qv table; tol 2e-2"):
                    nc.vector.tensor_scalar(
                        out=qv8[:], in0=qv_st[:], scalar1=F8_MAX,
                        scalar2=-F8_MAX, op0=OP.min, op1=OP.max,
                    )
                nc.sync.dma_start(qv_slice[l][t * P:(t + 1) * P, :], qv8[:])

            def emit_proj_tile(l, t):
                proj_pend.append((l, t, _proj_stage_a(l, t)))
                if len(proj_pend) > 1:
                    ll, tt, xT = proj_pend.pop(0)
                    _proj_stage_b(ll, tt, xT)

            def flush_proj():
                while proj_pend:
                    ll, tt, xT = proj_pend.pop(0)
                    _proj_stage_b(ll, tt, xT)

            def emit_ag(l, j):
                flush_proj()
                r0 = lay.tile_start[j] * P
                r1 = lay.tile_start[j + 1] * P
                g0 = int(lay.agbase[j])
                g1 = int(lay.agbase[j + 1])
                dst = (qv_lo[l][g0:g1, :] if g1 <= LO_ROWS
                       else qv_hi[l][g0 - LO_ROWS:g1 - LO_ROWS, :])
                nc.gpsimd.collective_compute(
                    "AllGather", OP.bypass, replica_groups=rg,
                    ins=[qv_slice[l][r0:r1, :]], outs=[dst],
                )

            def emit_w0(l, t0, tn):
                if l < L - 1:
                    nc.scalar.copy(
                        out=x_sb[:, t0 * D:(t0 + tn) * D].rearrange(
                            "p (t d) -> p t d", d=D),
                        in_=bo_sb[:, l * D:(l + 1) * D].unsqueeze(1)
                            .to_broadcast([P, tn, D]),
                    )
                else:
                    o_sb = projpool.tile([P, tn * nclass], FP, tag="o_sb0",
                                         name="o_sb0")
                    nc.scalar.copy(
                        out=o_sb[:].rearrange("p (t d) -> p t d", d=nclass),
                        in_=bo_sb[:, l * D:l * D + nclass].unsqueeze(1)
                            .to_broadcast([P, tn, nclass]),
                    )
                    nc.sync.dma_start(
                        out_ext[t0 * P:(t0 + tn) * P, :].rearrange(
                            "(t p) d -> p t d", p=P),
                        o_sb[:].rearrange("p (t d) -> p t d", d=nclass),
                    )

            # One SWDGE queue per group-half.  Tile's own DMA-completion
            # tracking is unsound here (its DMASW lane sems assume
            # single-queue FIFO completion, and gen_mode==1 preps tick on
            # desc-gen), so consumers carry explicit wait_op()s on the
            # per-queue DMA semaphore instead.
            GCH = 32  # slot-columns per gather (32*128 = 4096 idxs)

            def emit_half_gathers(l, g, hi):
                """Prep+trigger one half's gathers -> (tile, queue, sem_tgt).

                lo halves need only AG chunks 0-1, so the layer loop emits
                them a few groups ahead to keep Q7 desc-gen busy while the
                hi chunks' AllGather lands.
                """
                w = g.whi if hi else g.wlo
                cols_h = g.tiles * w
                if not cols_h:
                    return None
                pool, tag = (qvhpool, "qvgh") if hi else (qvlpool, "qvgl")
                gt = pool.tile([P, cols_h * 2 * D], F8, tag=tag, name=tag)
                q = qnext[0]
                qnext[0] = (q + 1) % NQ
                ioff = g.ioff_hi if hi else g.ioff_lo
                in_ap = (qv_hi[l][:, :] if hi else qv_lo[l][:, :])
                for cc in range(0, cols_h, GCH):
                    cn = min(GCH, cols_h - cc)
                    _chain_q(nc.gpsimd.dma_gather(
                        out_ap=gt[:, cc * 2 * D:(cc + cn) * 2 * D].rearrange(
                            "p (c e) -> p c e", e=2 * D),
                        in_ap=in_ap,
                        idxs_ap=idx_sb[:, ioff + 8 * cc:
                                       ioff + 8 * (cc + cn)],
                        num_idxs=P * cn, num_idxs_reg=P * cn,
                        elem_size=2 * D, queue_num=q,
                        prepare_only=True, sem=dma_sems[q],
                        single_packet=False,
                    ), q)
                    cum16[q] += 16
                _chain_q(nc.gpsimd.trigger_dma(count=None, queue_num=q), q)
                return (gt, q, cum16[q])

            def emit_group_compute(l, g, glo, ghi):
                Tg, wlo, whi = g.tiles, g.wlo, g.whi
                W = wlo + whi
                cols = Tg * W
                colsLo, colsHi = Tg * wlo, Tg * whi
                nt0 = g.nt0
                # fp8 -> fp16 upconvert on ACT (one copy per half; each
                # carries the explicit DMA-completion wait for its half)
                qv16 = qv16pool.tile([P, cols * 2 * D], F16, tag="qv16",
                                     name="qv16")
                if wlo:
                    gt, q, sem_tgt = glo
                    nc.scalar.copy(
                        out=qv16[:, :colsLo * 2 * D], in_=gt[:],
                    ).wait_op(dma_sems[q], sem_tgt, "sem-ge")
                if whi:
                    gt, q, sem_tgt = ghi
                    nc.scalar.copy(
                        out=qv16[:, colsLo * 2 * D:], in_=gt[:],
                    ).wait_op(dma_sems[q], sem_tgt, "sem-ge")
                kv = k_sb[:, nt0 * D:(nt0 + Tg) * D].rearrange(
                    "p (t d) -> p t d", d=D)

                # scores: tmp = q * k (broadcast over slots)
                tmp = tpool.tile([P, cols * D], F16, tag="tmp", name="tmp")
                if wlo:
                    nc.vector.tensor_tensor(
                        out=tmp[:, :colsLo * D].rearrange(
                            "p (t w d) -> p t w d", w=wlo, d=D),
                        in0=qv16[:, :colsLo * 2 * D].rearrange(
                            "p (t w e) -> p t w e", w=wlo, e=2 * D)[:, :, :, 0:D],
                        in1=kv.unsqueeze(2).to_broadcast([P, Tg, wlo, D]),
                        op=OP.mult,
                    )
                if whi:
                    nc.vector.tensor_tensor(
                        out=tmp[:, colsLo * D:].rearrange(
                            "p (t w d) -> p t w d", w=whi, d=D),
                        in0=qv16[:, colsLo * 2 * D:].rearrange(
                            "p (t w e) -> p t w e", w=whi, e=2 * D)[:, :, :, 0:D],
                        in1=kv.unsqueeze(2).to_broadcast([P, Tg, whi, D]),
                        op=OP.mult,
                    )

                # single reduce over d per (slot, head); fp16 out
                s_t = smpool.tile([P, cols * H], F16, tag="s", name="s")
                with nc.allow_low_precision("fp16 score sum; tol 2e-2"):
                    nc.vector.reduce_sum(
                        s_t[:],
                        tmp[:].rearrange("p (c h d) -> p c h d", h=H, d=HD),
                        axis=AX.X)

                # scores are small (|s| < 3): skip the max-subtract shift,
                # exponentiate directly (fp16-safe; pads masked after)
                ex = smpool.tile([P, cols * H], F16, tag="ex", name="ex")
                nc.scalar.activation(
                    out=ex[:], in_=s_t[:],
                    func=mybir.ActivationFunctionType.Exp)
                # mask: mask table is laid (t, w) with w in [0, W) combined
                for (c0, w0, wr) in ((0, 0, wlo), (colsLo, wlo, whi)):
                    if not wr:
                        continue
                    exv = ex[:, c0 * H:(c0 + Tg * wr) * H].rearrange(
                        "p (t w h) -> p t w h", w=wr, h=H)
                    mv = mask_sb[:, g.moff:g.moff + cols].rearrange(
                        "p (t w) -> p t w", w=W)[:, :, w0:w0 + wr]
                    nc.vector.tensor_tensor(
                        out=exv, in0=exv,
                        in1=mv.unsqueeze(3).to_broadcast([P, Tg, wr, H]),
                        op=OP.mult)

                # denom (fp32) + eps, rec = 1/denom
                ex_lo = ex[:, :colsLo * H].rearrange(
                    "p (t w h) -> p t h w", w=max(wlo, 1), h=H)
                ex_hi = ex[:, colsLo * H:].rearrange(
                    "p (t w h) -> p t h w", w=max(whi, 1), h=H)
                den = smpool.tile([P, Tg * H], FP, tag="den", name="den")
                if wlo and whi:
                    den2 = smpool.tile([P, Tg * H], FP, tag="den2", name="den2")
                    nc.vector.reduce_sum(den[:], ex_lo, axis=AX.X)
                    nc.vector.reduce_sum(den2[:], ex_hi, axis=AX.X)
                    nc.vector.scalar_tensor_tensor(
                        out=den[:], in0=den[:], scalar=DEN_EPS, in1=den2[:],
                        op0=OP.add, op1=OP.add)
                elif wlo:
                    nc.vector.reduce_sum(den[:], ex_lo, axis=AX.X)
                    nc.vector.tensor_scalar_add(out=den[:], in0=den[:],
                                                scalar1=DEN_EPS)
                else:
                    nc.vector.reduce_sum(den[:], ex_hi, axis=AX.X)
                    nc.vector.tensor_scalar_add(out=den[:], in0=den[:],
                                                scalar1=DEN_EPS)
                rec = smpool.tile([P, Tg * H], FP, tag="rec", name="rec")
                nc.vector.reciprocal(rec[:], den[:])

                # tmp2 = v * ex   (3 free dims: (t w) merged per half)
                tmp2 = tpool.tile([P, cols * D], F16, tag="tmp2", name="tmp2")
                for hi, (c0, wr) in enumerate(((0, wlo), (colsLo, whi))):
                    if not wr:
                        continue
                    ncols = Tg * wr
                    nc.vector.tensor_tensor(
                        out=tmp2[:, c0 * D:(c0 + ncols) * D].rearrange(
                            "p (c h d) -> p c h d", h=H, d=HD),
                        in0=qv16[:, c0 * 2 * D:(c0 + ncols) * 2 * D].rearrange(
                            "p (c e) -> p c e", e=2 * D)[:, :, D:2 * D].rearrange(
                            "p c (h d) -> p c h d", d=HD),
                        in1=ex[:, c0 * H:(c0 + ncols) * H].rearrange(
                            "p (c h) -> p c h", h=H).unsqueeze(3)
                            .to_broadcast([P, ncols, H, HD]),
                        op=OP.mult)

                # aggregate over w (pairwise tree), merge halves, normalize
                def lo_view(w0, w1):
                    return tmp2[:, :colsLo * D].rearrange(
                        "p (t w d) -> p t w d", w=wlo, d=D)[:, :, w0:w1, :]

                def hi_view(w0, w1):
                    return tmp2[:, colsLo * D:].rearrange(
                        "p (t w d) -> p t w d", w=whi, d=D)[:, :, w0:w1, :]

                aggN = apool.tile([P, Tg * D], F16, tag="aggN", name="aggN")
                aggU = apool.tile([P, Tg * D], F16, tag="aggU", name="aggU")
                aggUv = aggU[:].rearrange("p (t d) -> p t d", d=D).unsqueeze(2)
                if wlo and whi:
                    alo = _tree_reduce_w(nc, trpool, lo_view, Tg, wlo, "g")
                    nc.scalar.copy(out=aggUv, in_=alo)
                    ahi = _tree_reduce_w(nc, trpool, hi_view, Tg, whi, "g")
                    nc.vector.tensor_tensor(out=aggUv, in0=aggUv, in1=ahi,
                                            op=OP.add)
                else:
                    view = lo_view if wlo else hi_view
                    ww = wlo if wlo else whi
                    af = _tree_reduce_w(nc, trpool, view, Tg, ww, "g")
                    nc.scalar.copy(out=aggUv, in_=af)
                nc.vector.tensor_tensor(
                    out=aggN[:].rearrange("p (t h d) -> p t h d", h=H, d=HD),
                    in0=aggU[:].rearrange("p (t h d) -> p t h d", h=H, d=HD),
                    in1=rec[:].rearrange("p (t h) -> p t h", h=H).unsqueeze(3)
                        .to_broadcast([P, Tg, H, HD]),
                    op=OP.mult)

                # fused output projection per tile
                for ti in range(Tg):
                    nt = nt0 + ti
                    aT_ps = psA.tile([P, P], F16, tag="aT", name="aT")
                    nc.tensor.transpose(
                        aT_ps[:], aggN[:, ti * D:(ti + 1) * D], ident[:])
                    aT = projpool.tile([P, P], F16, tag="aT_sb", name="aT_sb")
                    nc.scalar.copy(out=aT[:], in_=aT_ps[:])
                    o_ps = psC.tile([P, D], FP, tag="o", name="o")
                    nc.tensor.matmul(
                        o_ps[:], lhsT=aT[:], rhs=wo_sb[:, l * D:(l + 1) * D],
                        start=True, stop=True,
                    )
                    if l < L - 1:
                        nc.vector.tensor_tensor(
                            out=x_sb[:, nt * D:(nt + 1) * D], in0=o_ps[:],
                            in1=bo_sb[:, l * D:(l + 1) * D], op=OP.add,
                        )
                    else:
                        o_sb = projpool.tile([P, nclass], FP, tag="o_sb",
                                             name="o_sb")
                        nc.vector.tensor_tensor(
                            out=o_sb[:], in0=o_ps[:, 0:nclass],
                            in1=bo_sb[:, l * D:l * D + nclass], op=OP.add,
                        )
                        nc.sync.dma_start(
                            out_ext[nt * P:(nt + 1) * P, :], o_sb[:])

            # edge-phase segments in tile order: w0 runs + gather groups
            segments = sorted(
                [("w0", t0, tn) for (t0, tn) in lay.w0_runs]
                + [("g", g.nt0, g) for g in lay.groups],
                key=lambda x: x[1])

            # layer-0 prologue: project + AllGather (nothing to overlap with)
            for j in range(NCH):
                for t in range(lay.tile_start[j], lay.tile_start[j + 1]):
                    emit_proj_tile(0, t)
                emit_ag(0, j)

            glist = [s[2] for s in segments if s[0] == "g"]
            KLO = 5  # lo-gather lookahead depth (groups)

            for l in range(L):
                nxt = l + 1
                next_chunk = 0
                projected = 0
                gj = 0
                pend_lo = {}
                if not SKIP_EDGE:
                    # lo-half gathers depend only on AG chunks 0-1: run a few
                    # ahead so Q7 stays busy while hi chunks' AG finishes
                    for j in range(min(KLO, len(glist))):
                        pend_lo[j] = emit_half_gathers(l, glist[j], hi=False)
                for seg in segments:
                    if seg[0] == "w0":
                        emit_w0(l, seg[1], seg[2])
                        done = seg[1] + seg[2]
                    else:
                        g = seg[2]
                        if not SKIP_EDGE:
                            if gj + KLO < len(glist):
                                pend_lo[gj + KLO] = emit_half_gathers(
                                    l, glist[gj + KLO], hi=False)
                            ghi = emit_half_gathers(l, g, hi=True)
                            glo = pend_lo.pop(gj)
                            if not GATHER_ONLY:
                                emit_group_compute(l, g, glo, ghi)
                        gj += 1
                        done = g.nt0 + g.tiles
                    # software-pipeline: project layer l+1 tiles whose edge
                    # output is complete; fire its AllGather chunks eagerly
                    while (nxt < L and next_chunk < NCH
                           and lay.tile_start[next_chunk + 1] <= done):
                        for t in range(projected,
                                       lay.tile_start[next_chunk + 1]):
                            emit_proj_tile(nxt, t)
                        projected = lay.tile_start[next_chunk + 1]
                        emit_ag(nxt, next_chunk)
                        next_chunk += 1
                while nxt < L and next_chunk < NCH:
                    for t in range(projected, lay.tile_start[next_chunk + 1]):
                        emit_proj_tile(nxt, t)
                    projected = lay.tile_start[next_chunk + 1]
                    emit_ag(nxt, next_chunk)
                    next_chunk += 1
    nc.compile()
    return nc


# ----------------------------------------------------------------------------
# Entry point
# ----------------------------------------------------------------------------

_trace = [False]  # test.py can flip this to profile


def kernel(x, src, dst, qkv_w, qkv_b, out_w, out_b, out_w_last, out_b_last):
    x = np.asarray(x, dtype=np.float32)
    lay = build_layout(np.asarray(src), np.asarray(dst), x.shape[0])
    nclass = np.asarray(out_w_last).shape[1]
    in_maps = host_inputs(
        lay, x, np.asarray(qkv_w, dtype=np.float32),
        np.asarray(qkv_b, dtype=np.float32), np.asarray(out_w, dtype=np.float32),
        np.asarray(out_b, dtype=np.float32), np.asarray(out_w_last, dtype=np.float32),
        np.asarray(out_b_last, dtype=np.float32),
    )
    nc = build_nc(lay, nclass)
    res = run_bass_kernel_spmd(
        nc, in_maps, core_ids=list(range(NCORES)), trace=_trace[0]
    )
    kernel.last_results = res
    outs = [res.results[c]["out"] for c in range(NCORES)]
    return host_output(lay, outs, nclass)

